# revision 7
# baseline (speedup 1.0000x reference)
"""Trainium2 Bass kernel for nn_BatchdenseGAT: 2-layer dense GAT, batch 16x512 nodes.

Strategy: pure data-parallel over the batch dim -- 2 graphs per NeuronCore, 8 cores.
Each core runs embedding gather + instance norms + 2 GAT layers + log_softmax for its
2 graphs. No collectives.

v2 notes (vs baseline):
  - All activations/weights pre-cast to bf16 on host; h and user_emb pre-transposed
    on host; weights packed into one [128, C] DRAM tensor; -colsum(w1) (the elu+1
    bias fold) precomputed on host. No device-side casts / weight staging.
  - adj transposed by the DMA XBAR engine (dma_start_transpose) straight from DRAM,
    eliminating 32 fp32 PE transposes + psum->sbuf copies per core.
  - attention s/d rows: d-row is redistributed into per-chunk columns with one small
    SBUF->SBUF DMA instead of 4 PE transposes per (g,h,layer).
  - software pipeline depth 2 (A(i) ... EB(i-1) ... B(i-2)) to hide the dcol DMA
    latency and keep the tensor engine dense (p-state ramp).
  - DMA triggers ride the sync queue; elementwise work balanced across DVE/ACT/GPSIMD.

Layout (per graph, per head):
  - x^T [131, 512] feature-major feeds matmuls as the moving operand.
  - hp^T [128, 512] from weights-stationary matmuls.
  - attention built in TRANSPOSED form E'[m, n] (m=src, n=dst); E' blocks are the
    moving operand of attn@hp with contraction over m on partitions.
  - exp(leaky_relu(s[n]+d[m], .2)) = e^{.2s[n]} * max(e^{.8s[n]}e^{d[m]}, e^{.2d[m]});
    the e^{.2s[n]} column factor is softmax-invariant and dropped.
  - softmax denominators via ones-column matmuls; normalization in epilogues.
  - elu(z)+1 = min(exp(z), relu(z)+1); the -1 is folded into layer 1 via the
    host-computed -colsum(w1) bias.
"""

import os
import sys
import numpy as np

B, N, V, D_EMB, F0, H = 16, 512, 100000, 64, 64, 8
O1 = O2 = 128
F_IN0 = F0 + D_EMB + 3  # 131
F_IN1 = H * O1          # 1024
EPS = 1e-5
NCORES = 8
G = B // NCORES         # graphs per core = 2
NCH = N // 128          # 4 node chunks

# packed weight tensor column offsets (bf16, 128 partitions)
W0A_OFF = 0                      # 8 heads x 128 cols
W1_OFF = W0A_OFF + H * 128       # 8 heads x 8 ktiles x 128 cols
A_OFF = W1_OFF + H * 1024        # 2 layers x 8 heads x 2 cols (src,dst)
WPACK_COLS = A_OFF + 4 * H

_cache = {}


def _ensure_paths():
    p = "/opt/trn_rl_repo/concourse"
    if os.path.isdir(p) and p not in sys.path:
        sys.path.append(p)


def _build_nc():
    _ensure_paths()
    import concourse.bass as bass
    import concourse.tile as tile
    import concourse.mybir as mybir
    from concourse import bacc
    from concourse.masks import make_identity
    from contextlib import ExitStack

    F32 = mybir.dt.float32
    BF16 = mybir.dt.bfloat16
    I32 = mybir.dt.int32
    AX = mybir.AxisListType
    OP = mybir.AluOpType
    ACT = mybir.ActivationFunctionType

    nc = bacc.Bacc("TRN2", debug=False, enable_asserts=False)

    d_vert = nc.dram_tensor("vertices", [G, N, 1], I32, kind="ExternalInput").ap()
    d_adj = nc.dram_tensor("adj", [G, N, N], BF16, kind="ExternalInput").ap()
    d_hT = nc.dram_tensor("hT", [G, F0, N], BF16, kind="ExternalInput").ap()
    d_ueT = nc.dram_tensor("ueT", [G, 3, N], BF16, kind="ExternalInput").ap()
    d_emb = nc.dram_tensor("emb", [V, D_EMB], BF16, kind="ExternalInput").ap()
    d_wpack = nc.dram_tensor("wpack", [128, WPACK_COLS], BF16, kind="ExternalInput").ap()
    d_w0b = nc.dram_tensor("w0b", [3, H * 128], BF16, kind="ExternalInput").ap()
    d_ncs = nc.dram_tensor("ncs", [128, H], F32, kind="ExternalInput").ap()
    d_n1wb = nc.dram_tensor("n1wb", [D_EMB, 2], F32, kind="ExternalInput").ap()
    d_n2wb = nc.dram_tensor("n2wb", [3, 2], F32, kind="ExternalInput").ap()
    d_out = nc.dram_tensor("out", [G, N, O2], F32, kind="ExternalOutput").ap()

    with tile.TileContext(nc) as tc, ExitStack() as ctx:
        pers = ctx.enter_context(tc.tile_pool(name="pers", bufs=1))
        wk = ctx.enter_context(tc.tile_pool(name="wk", bufs=2))
        ep_pool = ctx.enter_context(tc.tile_pool(name="ep", bufs=12))
        psm = ctx.enter_context(tc.tile_pool(name="psm", bufs=2, space="PSUM"))
        pst = ctx.enter_context(tc.tile_pool(name="pst", bufs=2, space="PSUM"))
        pss = ctx.enter_context(tc.tile_pool(name="pss", bufs=2, space="PSUM"))

        _cp = [0]

        def anycopy(out, in_):
            # alternate ACT/DVE for psum->sbuf copies (GPSIMD cannot read PSUM)
            _cp[0] ^= 1
            if _cp[0]:
                nc.scalar.copy(out, in_)
            else:
                nc.vector.tensor_copy(out=out, in_=in_)

        MM = nc.tensor.matmul

        # ---------- persistents ----------
        ident_b = pers.tile([128, 128], BF16, tag="ident_b")
        make_identity(nc, ident_b[:])
        ident = pers.tile([128, 128], F32, tag="ident")
        make_identity(nc, ident[:])

        ones_b = pers.tile([128, 1], BF16, tag="ones_b")
        nc.vector.memset(ones_b[:], 1.0)
        eights_b = pers.tile([128, 1], BF16, tag="eights_b")
        nc.vector.memset(eights_b[:], float(H))

        # packed weights -> SBUF directly (bf16, no staging/cast)
        wpack = pers.tile([128, WPACK_COLS], BF16, tag="wpack")
        NSPLIT = 4
        cw = WPACK_COLS // NSPLIT
        for s in range(NSPLIT):
            nc.sync.dma_start(wpack[:, cw * s:cw * (s + 1)],
                              d_wpack[:, cw * s:cw * (s + 1)])
        w0b_r = pers.tile([3, H * 128], BF16, tag="w0b_r")
        nc.sync.dma_start(w0b_r[:], d_w0b[:])
        negcs1 = pers.tile([128, H], F32, tag="negcs1")
        nc.sync.dma_start(negcs1[:], d_ncs[:])
        n1wb = pers.tile([D_EMB, 2], F32, tag="n1wb")
        nc.sync.dma_start(n1wb[:], d_n1wb[:])
        n2wb = pers.tile([3, 2], F32, tag="n2wb")
        nc.sync.dma_start(n2wb[:], d_n2wb[:])

        def w0a_h(h):
            return wpack[:, W0A_OFF + 128 * h:W0A_OFF + 128 * (h + 1)]

        def w1_hk(h, k):
            base = W1_OFF + 1024 * h + 128 * k
            return wpack[:, base:base + 128]

        def a_pair(layer, h):
            base = A_OFF + 2 * (H * layer + h)
            return wpack[:, base:base + 2]

        # adj^T via DMA XBAR transpose: adjT_all[g][p, 512*j + n] = adj[g][n, 128j+p]
        adjT_all = [pers.tile([128, NCH * N], BF16, tag=f"adjT{g}", name=f"adjT{g}")
                    for g in range(G)]
        xTa = [pers.tile([128, N], BF16, tag=f"xTa{g}", name=f"xTa{g}") for g in range(G)]
        xTb = [pers.tile([3, N], BF16, tag=f"xTb{g}", name=f"xTb{g}") for g in range(G)]
        x1T = [[pers.tile([128, N], BF16, tag=f"x1T{g}_{h}", name=f"x1T{g}_{h}") for h in range(H)]
               for g in range(G)]
        acc = [pers.tile([128, N], F32, tag=f"acc{g}", name=f"acc{g}") for g in range(G)]

        def adjT(g, j):
            return adjT_all[g][:, N * j:N * (j + 1)]

        # ---------- stage 1: per-graph preprocessing ----------
        def instance_norm_to(dst, src, P, wb_col):
            # dst[P,N] = (src - mu) * rstd * w + b, stats over free dim
            s1 = wk.tile([P, 1], F32, tag="in_sum")
            nc.vector.tensor_reduce(s1[:], src[:], AX.X, OP.add)
            sq = wk.tile([P, N], BF16, tag="in_sq", name="sq")
            ssq = wk.tile([P, 1], F32, tag="in_ssq")
            nc.scalar.activation(sq[:], src[:], ACT.Square, accum_out=ssq[:])
            mu = wk.tile([P, 1], F32, tag="in_mu")
            nc.vector.tensor_scalar(mu[:], s1[:], 1.0 / N, None, OP.mult)
            ex2 = wk.tile([P, 1], F32, tag="in_ex2")
            nc.vector.tensor_scalar(ex2[:], ssq[:], 1.0 / N, None, OP.mult)
            musq = wk.tile([P, 1], F32, tag="in_musq")
            nc.vector.tensor_tensor(out=musq[:], in0=mu[:], in1=mu[:], op=OP.mult)
            var = wk.tile([P, 1], F32, tag="in_var")
            nc.vector.tensor_tensor(out=var[:], in0=ex2[:], in1=musq[:], op=OP.subtract)
            vare = wk.tile([P, 1], F32, tag="in_vare")
            nc.vector.tensor_scalar(vare[:], var[:], EPS, None, OP.add)
            # rsqrt via quake seed + 3 Newton iterations, all on DVE
            iv = vare[:].bitcast(mybir.dt.int32)
            sh = wk.tile([P, 1], mybir.dt.int32, tag="in_sh")
            nc.vector.tensor_scalar(sh[:], iv, 1, None, OP.arith_shift_right)
            y = wk.tile([P, 1], F32, tag="in_y")
            nc.vector.tensor_scalar(y[:].bitcast(mybir.dt.int32), sh[:], -1,
                                    0x5f3759df, OP.mult, OP.add)
            rstd = y
            for it in range(3):
                y2 = wk.tile([P, 1], F32, tag="in_y2", name="y2")
                nc.vector.tensor_tensor(out=y2[:], in0=rstd[:], in1=rstd[:], op=OP.mult)
                vy2 = wk.tile([P, 1], F32, tag="in_vy2", name="vy2")
                nc.vector.tensor_tensor(out=vy2[:], in0=vare[:], in1=y2[:], op=OP.mult)
                corr = wk.tile([P, 1], F32, tag="in_corr", name="corr")
                nc.vector.tensor_scalar(corr[:], vy2[:], -0.5, 1.5, OP.mult, OP.add)
                ynew = wk.tile([P, 1], F32, tag="in_ynew", name="ynew")
                nc.vector.tensor_tensor(out=ynew[:], in0=rstd[:], in1=corr[:], op=OP.mult)
                rstd = ynew
            scl = wk.tile([P, 1], F32, tag="in_scl")
            nc.vector.tensor_tensor(out=scl[:], in0=rstd[:], in1=wb_col[:, 0:1], op=OP.mult)
            tb = wk.tile([P, 1], F32, tag="in_tb")
            nc.vector.tensor_tensor(out=tb[:], in0=mu[:], in1=scl[:], op=OP.mult)
            bia = wk.tile([P, 1], F32, tag="in_bia")
            nc.vector.tensor_tensor(out=bia[:], in0=wb_col[:, 1:2], in1=tb[:], op=OP.subtract)
            nc.vector.tensor_scalar(dst, src[:], scl[:], bia[:], OP.mult, OP.add)

        def prep_graph(g):
            # adj^T straight from DRAM via the DMA XBAR
            nc.sync.dma_start_transpose(
                adjT_all[g][:].rearrange("p (j n) -> p j n", j=NCH), d_adj[g])
            # h^T pre-transposed on host -> xTa rows 0:64
            nc.sync.dma_start(xTa[g][0:F0, :], d_hT[g])
            # embedding gather -> transpose -> instance norm -> xTa rows 64:128
            embT = wk.tile([D_EMB, N], BF16, tag="embT", name="embT")
            for i in range(NCH):
                idx = wk.tile([128, 1], I32, tag="idx")
                nc.gpsimd.dma_start(idx[:], d_vert[g, 128 * i:128 * (i + 1), :])
                gat = wk.tile([128, D_EMB], BF16, tag="gat")
                nc.gpsimd.indirect_dma_start(
                    out=gat[:], out_offset=None, in_=d_emb[:],
                    in_offset=bass.IndirectOffsetOnAxis(ap=idx[:, :1], axis=0))
                tp = pst.tile([128, 128], BF16, tag="tpb", name="prep_tp")
                nc.tensor.transpose(tp[0:D_EMB, :], gat[:], ident_b[:])
                anycopy(embT[:, 128 * i:128 * (i + 1)], tp[0:D_EMB, :])
            instance_norm_to(xTa[g][F0:F0 + D_EMB, :], embT, D_EMB, n1wb)
            # user_emb^T pre-transposed on host -> instance norm -> xTb
            ueT = wk.tile([3, N], BF16, tag="ueT", name="ueT")
            nc.sync.dma_start(ueT[:], d_ueT[g])
            instance_norm_to(xTb[g][:], ueT, 3, n2wb)

        # ---------- pipelined GAT layer machinery ----------
        # stage A(i): hp matmuls, tanh, hp^T->hpblk transposes, s/d row matmul,
        #             d-row->column DMA, p_row exp + broadcast
        # stage EB(i): exps of d columns, E' tile construction (DVE/GPSIMD)
        # stage B(i):  softmax denominator matmul, value matmuls, epilogue

        def stageA(layer, g, h):
            hp_ps = psm.tile([128, N], F32, tag="mm", name="hp_ps")
            if layer == 0:
                MM(hp_ps[:], w0a_h(h), xTa[g][:], start=True, stop=False)
                MM(hp_ps[:], w0b_r[:, 128 * h:128 * (h + 1)], xTb[g][:],
                   start=False, stop=True)
            else:
                for k in range(8):
                    MM(hp_ps[:], w1_hk(h, k), x1T[g][k][:],
                       start=(k == 0), stop=(k == 7))
            t0 = wk.tile([128, N], BF16, tag="tt", name="t0")
            hp_sb = wk.tile([128, N], BF16, tag="hpsb", name="hp_sb", bufs=2)
            if layer == 0:
                nc.scalar.activation(t0[:], hp_ps[:], ACT.Tanh)
                nc.vector.tensor_copy(out=hp_sb[:], in_=hp_ps[:])
            else:
                ncs = negcs1[:, h:h + 1]
                nc.scalar.activation(t0[:], hp_ps[:], ACT.Tanh, bias=ncs)
                nc.vector.tensor_scalar(hp_sb[:], hp_ps[:], ncs, None, OP.add)
            hpblk = wk.tile([128, N], BF16, tag="hpblk", name="hpblk", bufs=3)
            for j in range(NCH):
                tp = pst.tile([128, 128], BF16, tag="tpb")
                nc.tensor.transpose(tp[:], hp_sb[:, 128 * j:128 * (j + 1)], ident_b[:])
                anycopy(hpblk[:, 128 * j:128 * (j + 1)], tp[:])
            # s/d rows: [s; d] = a^T @ tanh(hp^T)
            s_ps = pss.tile([2, N], F32, tag="sd", name="s_ps")
            MM(s_ps[:], a_pair(layer, h), t0[:], start=True, stop=True)
            sd_sb = wk.tile([2, N], F32, tag="sdsb", name="sd_sb", bufs=3)
            nc.scalar.copy(sd_sb[:], s_ps[:])
            # d-row -> per-chunk columns: dcol[p, j] = d[128j + p]
            dcol = wk.tile([128, NCH], F32, tag="dcol", name="dcol", bufs=4)
            for j in range(NCH):
                nc.sync.dma_start(dcol[:, j:j + 1], sd_sb[1:2, 128 * j:128 * (j + 1)])
            # softmax-row-invariant factor e^{0.8 s[n]} broadcast over partitions
            p_row = wk.tile([1, N], BF16, tag="prow", bufs=2)
            nc.scalar.activation(p_row[:], s_ps[0:1, :], ACT.Exp, scale=0.8)
            p_b = wk.tile([128, N], BF16, tag="pb", bufs=3)
            nc.gpsimd.partition_broadcast(p_b[:], p_row[:])
            return dict(g=g, h=h, layer=layer, hpblk=hpblk, dcol=dcol, p_b=p_b)

        def stageEB(st):
            g = st["g"]
            dcol, p_b = st["dcol"], st["p_b"]
            acol = wk.tile([128, NCH], F32, tag="acol", bufs=2)
            nc.scalar.activation(acol[:], dcol[:], ACT.Exp, scale=0.2)
            qacol = wk.tile([128, NCH], F32, tag="qacol", bufs=2)
            nc.scalar.activation(qacol[:], dcol[:], ACT.Exp, scale=1.0)
            eps_ = []
            for j in range(NCH):
                e = ep_pool.tile([128, N], BF16, tag="ep", name="e")
                nc.vector.tensor_scalar(e[:], p_b[:], qacol[:, j:j + 1],
                                        acol[:, j:j + 1], OP.mult, OP.max)
                if j < 3:
                    nc.vector.tensor_tensor(out=e[:], in0=e[:], in1=adjT(g, j),
                                            op=OP.mult)
                else:
                    nc.gpsimd.tensor_tensor(out=e[:], in0=e[:], in1=adjT(g, j),
                                            op=OP.mult)
                eps_.append(e)
            st["eps"] = eps_
            return st

        def stageB(st):
            g, h, layer = st["g"], st["h"], st["layer"]
            hpblk, eps_ = st["hpblk"], st["eps"]
            rs_ps = pss.tile([1, N], F32, tag="sd", name="rs_ps")
            colw = ones_b if layer == 0 else eights_b
            for j in range(NCH):
                MM(rs_ps[:], colw[:], eps_[j][:],
                   start=(j == 0), stop=(j == NCH - 1))
            rrec = wk.tile([1, N], F32, tag="rrec")
            nc.vector.reciprocal_approx_fast(out=rrec[:], in_=rs_ps[:])
            rb = wk.tile([128, N], F32, tag="rb")
            nc.gpsimd.partition_broadcast(rb[:], rrec[:])
            out_ps = psm.tile([128, N], F32, tag="mm", name="out_ps")
            for j in range(NCH):
                MM(out_ps[:], hpblk[:, 128 * j:128 * (j + 1)], eps_[j][:],
                   start=(j == 0), stop=(j == NCH - 1))
            if layer == 0:
                z = wk.tile([128, N], F32, tag="z")
                nc.vector.tensor_tensor(out=z[:], in0=out_ps[:], in1=rb[:], op=OP.mult)
                # elu(z)+1 = min(exp(z), relu(z)+1); -1 folded into layer-1 bias
                ez = wk.tile([128, N], F32, tag="ez")
                nc.scalar.activation(ez[:], z[:], ACT.Exp)
                r1 = wk.tile([128, N], F32, tag="r1")
                nc.vector.tensor_scalar(r1[:], z[:], 0.0, 1.0, OP.max, OP.add)
                nc.vector.tensor_tensor(out=x1T[g][h][:], in0=ez[:], in1=r1[:],
                                        op=OP.min)
            else:
                if h == 0:
                    nc.vector.tensor_tensor(out=acc[g][:], in0=out_ps[:], in1=rb[:],
                                            op=OP.mult)
                else:
                    contrib = wk.tile([128, N], F32, tag="contrib")
                    nc.vector.tensor_tensor(out=contrib[:], in0=out_ps[:], in1=rb[:],
                                            op=OP.mult)
                    nc.vector.tensor_tensor(out=acc[g][:], in0=acc[g][:],
                                            in1=contrib[:], op=OP.add)

        # ---------- run the pipeline across both layers ----------
        prep_graph(0)
        steps = [(l, g, h) for l in range(2) for g in range(G) for h in range(H)]
        window = []  # [(st, has_eb)]
        first = True
        for (l, g, h) in steps:
            st = stageA(l, g, h)
            if first:
                prep_graph(1)  # overlaps with layer-0 compute on graph 0
                first = False
            if len(window) >= 1:
                stageEB(window[-1][0])
            if len(window) >= 2:
                stageB(window[0][0])
                window.pop(0)
            window.append([st, False])
        stageEB(window[-1][0])
        for w in window:
            stageB(w[0])

        # ---------- log_softmax + output ----------
        ztps, nmaxs, sexps = [], [], []
        for g in range(G):
            for j in range(NCH):
                ztp = psm.tile([128, 128], F32, tag="mm", name="ztp")
                nc.tensor.transpose(ztp[:], acc[g][:, 128 * j:128 * (j + 1)], ident[:])
                zsb = wk.tile([128, 128], F32, tag="zsb", bufs=8, name="zsb")
                nc.vector.tensor_copy(out=zsb[:], in_=ztp[:])
                nmax = wk.tile([128, 1], F32, tag="nmax", bufs=8, name="nmax")
                nc.vector.tensor_reduce(nmax[:], zsb[:], AX.X, OP.max, negate=True)
                esc = wk.tile([128, 128], F32, tag="esc", name="esc")
                sexp = wk.tile([128, 1], F32, tag="sexp", bufs=8, name="sexp")
                nc.scalar.activation(esc[:], zsb[:], ACT.Exp, bias=nmax[:],
                                     accum_out=sexp[:])
                ztps.append(zsb); nmaxs.append(nmax); sexps.append(sexp)
        fins = []
        for i in range(G * NCH):
            lns = wk.tile([128, 1], F32, tag="lns", bufs=8, name="lns")
            nc.scalar.activation(lns[:], sexps[i][:], ACT.Ln)
            cc = wk.tile([128, 1], F32, tag="cc", bufs=8, name="cc")
            nc.vector.tensor_tensor(out=cc[:], in0=nmaxs[i][:], in1=lns[:],
                                    op=OP.subtract)
            fin = wk.tile([128, 128], F32, tag="fin", bufs=4, name="fin")
            nc.vector.tensor_scalar(fin[:], ztps[i][:], cc[:], None, OP.add)
            fins.append(fin)
        i = 0
        for g in range(G):
            for j in range(NCH):
                nc.sync.dma_start(d_out[g, 128 * j:128 * (j + 1), :], fins[i][:])
                i += 1

    nc.finalize()
    return nc


def _get_nc():
    if "nc" not in _cache:
        _cache["nc"] = _build_nc()
    return _cache["nc"]


def shard_inputs(inputs):
    """Full inputs -> list of 8 per-core input maps (host-side layout+cast)."""
    import ml_dtypes
    bf16 = ml_dtypes.bfloat16

    vertices = np.asarray(inputs["vertices"]).astype(np.int32).reshape(B, N, 1)
    adj = np.asarray(inputs["adj"], dtype=np.float32).astype(bf16)
    hT = np.ascontiguousarray(
        np.asarray(inputs["h"], dtype=np.float32).transpose(0, 2, 1)).astype(bf16)
    ueT = np.ascontiguousarray(
        np.asarray(inputs["user_emb"], dtype=np.float32).transpose(0, 2, 1)).astype(bf16)
    emb = np.asarray(inputs["emb_table"], dtype=np.float32).astype(bf16)

    w0 = np.asarray(inputs["w0"], dtype=np.float32)       # [H, 131, 128]
    w1 = np.asarray(inputs["w1"], dtype=np.float32)       # [H, 1024, 128]
    a0s = np.asarray(inputs["a_src0"], dtype=np.float32)[..., 0]  # [H, 128]
    a0d = np.asarray(inputs["a_dst0"], dtype=np.float32)[..., 0]
    a1s = np.asarray(inputs["a_src1"], dtype=np.float32)[..., 0]
    a1d = np.asarray(inputs["a_dst1"], dtype=np.float32)[..., 0]

    wpack = np.zeros((128, WPACK_COLS), dtype=np.float32)
    for h in range(H):
        wpack[:, W0A_OFF + 128 * h:W0A_OFF + 128 * (h + 1)] = w0[h, :128, :]
        for k in range(8):
            base = W1_OFF + 1024 * h + 128 * k
            wpack[:, base:base + 128] = w1[h, 128 * k:128 * (k + 1), :]
        wpack[:, A_OFF + 2 * h] = a0s[h]
        wpack[:, A_OFF + 2 * h + 1] = a0d[h]
        wpack[:, A_OFF + 2 * H + 2 * h] = a1s[h]
        wpack[:, A_OFF + 2 * H + 2 * h + 1] = a1d[h]
    wpack = wpack.astype(bf16)
    w0b = np.zeros((3, H * 128), dtype=np.float32)
    for h in range(H):
        w0b[:, 128 * h:128 * (h + 1)] = w0[h, 128:, :]
    w0b = w0b.astype(bf16)

    # elu fold bias: x1 is stored as elu(.)+1, so layer-1 needs -colsum(w1) per head.
    # matmul runs in bf16, so compute the colsum from the bf16-rounded weights.
    ncs = -w1.astype(bf16).astype(np.float32).sum(axis=1).transpose(1, 0)  # [128, H]
    ncs = np.ascontiguousarray(ncs, dtype=np.float32)

    n1wb = np.stack([np.asarray(inputs["norm1_w"], dtype=np.float32),
                     np.asarray(inputs["norm1_b"], dtype=np.float32)], axis=1)
    n2wb = np.stack([np.asarray(inputs["norm2_w"], dtype=np.float32),
                     np.asarray(inputs["norm2_b"], dtype=np.float32)], axis=1)

    maps = []
    for c in range(NCORES):
        sl = slice(G * c, G * (c + 1))
        maps.append({
            "vertices": np.ascontiguousarray(vertices[sl]),
            "adj": np.ascontiguousarray(adj[sl]),
            "hT": np.ascontiguousarray(hT[sl]),
            "ueT": np.ascontiguousarray(ueT[sl]),
            "emb": emb,
            "wpack": wpack, "w0b": w0b, "ncs": ncs,
            "n1wb": n1wb, "n2wb": n2wb,
        })
    return maps


def kernel(**inputs):
    _ensure_paths()
    from concourse import bass_utils
    nc = _get_nc()
    maps = shard_inputs(inputs)
    res = bass_utils.run_bass_kernel_spmd(nc, maps, core_ids=list(range(NCORES)))
    out = np.concatenate([res.results[c]["out"] for c in range(NCORES)], axis=0)
    return out


# revision 9
# speedup vs baseline: 1.0158x; 1.0158x over previous
"""Trainium2 Bass kernel for nn_BatchdenseGAT: 2-layer dense GAT, batch 16x512 nodes.

Strategy: pure data-parallel over the batch dim -- 2 graphs per NeuronCore, 8 cores.
Each core runs embedding gather + instance norms + 2 GAT layers + log_softmax for its
2 graphs. No collectives.

v2 notes (vs baseline):
  - All activations/weights pre-cast to bf16 on host; h and user_emb pre-transposed
    on host; weights packed into one [128, C] DRAM tensor; -colsum(w1) (the elu+1
    bias fold) precomputed on host. No device-side casts / weight staging.
  - adj transposed by the DMA XBAR engine (dma_start_transpose) straight from DRAM,
    eliminating 32 fp32 PE transposes + psum->sbuf copies per core.
  - attention s/d rows: d-row is redistributed into per-chunk columns with one small
    SBUF->SBUF DMA instead of 4 PE transposes per (g,h,layer).
  - software pipeline depth 2 (A(i) ... EB(i-1) ... B(i-2)) to hide the dcol DMA
    latency and keep the tensor engine dense (p-state ramp).
  - DMA triggers ride the sync queue; elementwise work balanced across DVE/ACT/GPSIMD.

Layout (per graph, per head):
  - x^T [131, 512] feature-major feeds matmuls as the moving operand.
  - hp^T [128, 512] from weights-stationary matmuls.
  - attention built in TRANSPOSED form E'[m, n] (m=src, n=dst); E' blocks are the
    moving operand of attn@hp with contraction over m on partitions.
  - exp(leaky_relu(s[n]+d[m], .2)) = e^{.2s[n]} * max(e^{.8s[n]}e^{d[m]}, e^{.2d[m]});
    the e^{.2s[n]} column factor is softmax-invariant and dropped.
  - softmax denominators via ones-column matmuls; normalization in epilogues.
  - elu(z)+1 = min(exp(z), relu(z)+1); the -1 is folded into layer 1 via the
    host-computed -colsum(w1) bias.
"""

import os
import sys
import numpy as np

B, N, V, D_EMB, F0, H = 16, 512, 100000, 64, 64, 8
O1 = O2 = 128
F_IN0 = F0 + D_EMB + 3  # 131
F_IN1 = H * O1          # 1024
EPS = 1e-5
NCORES = 8
G = B // NCORES         # graphs per core = 2
NCH = N // 128          # 4 node chunks

# packed weight tensor column offsets (bf16, 128 partitions)
W0A_OFF = 0                      # 8 heads x 128 cols
W1_OFF = W0A_OFF + H * 128       # 8 heads x 8 ktiles x 128 cols
A_OFF = W1_OFF + H * 1024        # 2 layers x 8 heads x 2 cols (src,dst)
WPACK_COLS = A_OFF + 4 * H

_cache = {}


def _ensure_paths():
    p = "/opt/trn_rl_repo/concourse"
    if os.path.isdir(p) and p not in sys.path:
        sys.path.append(p)


def _build_nc():
    _ensure_paths()
    import concourse.bass as bass
    import concourse.tile as tile
    import concourse.mybir as mybir
    from concourse import bacc
    from concourse.masks import make_identity
    from contextlib import ExitStack

    F32 = mybir.dt.float32
    BF16 = mybir.dt.bfloat16
    I32 = mybir.dt.int32
    AX = mybir.AxisListType
    OP = mybir.AluOpType
    ACT = mybir.ActivationFunctionType

    nc = bacc.Bacc("TRN2", debug=False, enable_asserts=False)

    d_vert = nc.dram_tensor("vertices", [G, N, 1], I32, kind="ExternalInput").ap()
    d_adj = nc.dram_tensor("adj", [G, N, N], BF16, kind="ExternalInput").ap()
    d_hT = nc.dram_tensor("hT", [G, F0, N], BF16, kind="ExternalInput").ap()
    d_ueT = nc.dram_tensor("ueT", [G, 3, N], BF16, kind="ExternalInput").ap()
    d_emb = nc.dram_tensor("emb", [V, D_EMB], BF16, kind="ExternalInput").ap()
    d_wpack = nc.dram_tensor("wpack", [128, WPACK_COLS], BF16, kind="ExternalInput").ap()
    d_w0b = nc.dram_tensor("w0b", [3, H * 128], BF16, kind="ExternalInput").ap()
    d_ncs = nc.dram_tensor("ncs", [128, H], F32, kind="ExternalInput").ap()
    d_n1wb = nc.dram_tensor("n1wb", [D_EMB, 2], F32, kind="ExternalInput").ap()
    d_n2wb = nc.dram_tensor("n2wb", [3, 2], F32, kind="ExternalInput").ap()
    d_out = nc.dram_tensor("out", [G, N, O2], F32, kind="ExternalOutput").ap()

    with tile.TileContext(nc) as tc, ExitStack() as ctx:
        pers = ctx.enter_context(tc.tile_pool(name="pers", bufs=1))
        wk = ctx.enter_context(tc.tile_pool(name="wk", bufs=2))
        ep_pool = ctx.enter_context(tc.tile_pool(name="ep", bufs=12))
        psm = ctx.enter_context(tc.tile_pool(name="psm", bufs=2, space="PSUM"))
        pst = ctx.enter_context(tc.tile_pool(name="pst", bufs=2, space="PSUM"))
        pss = ctx.enter_context(tc.tile_pool(name="pss", bufs=2, space="PSUM"))

        _cp = [0]

        def anycopy(out, in_):
            # alternate ACT/DVE for psum->sbuf copies (GPSIMD cannot read PSUM)
            _cp[0] ^= 1
            if _cp[0]:
                nc.scalar.copy(out, in_)
            else:
                nc.vector.tensor_copy(out=out, in_=in_)

        MM = nc.tensor.matmul

        # ---------- persistents ----------
        ident_b = pers.tile([128, 128], BF16, tag="ident_b")
        make_identity(nc, ident_b[:])
        ident = pers.tile([128, 128], F32, tag="ident")
        make_identity(nc, ident[:])

        ones_b = pers.tile([128, 1], BF16, tag="ones_b")
        nc.vector.memset(ones_b[:], 1.0)
        eights_b = pers.tile([128, 1], BF16, tag="eights_b")
        nc.vector.memset(eights_b[:], float(H))

        # packed weights -> SBUF directly (bf16, no staging/cast)
        wpack = pers.tile([128, WPACK_COLS], BF16, tag="wpack")
        NSPLIT = 4
        cw = WPACK_COLS // NSPLIT
        for s in range(NSPLIT):
            nc.sync.dma_start(wpack[:, cw * s:cw * (s + 1)],
                              d_wpack[:, cw * s:cw * (s + 1)])
        w0b_r = pers.tile([3, H * 128], BF16, tag="w0b_r")
        nc.sync.dma_start(w0b_r[:], d_w0b[:])
        negcs1 = pers.tile([128, H], F32, tag="negcs1")
        nc.sync.dma_start(negcs1[:], d_ncs[:])
        n1wb = pers.tile([D_EMB, 2], F32, tag="n1wb")
        nc.sync.dma_start(n1wb[:], d_n1wb[:])
        n2wb = pers.tile([3, 2], F32, tag="n2wb")
        nc.sync.dma_start(n2wb[:], d_n2wb[:])

        def w0a_h(h):
            return wpack[:, W0A_OFF + 128 * h:W0A_OFF + 128 * (h + 1)]

        def w1_hk(h, k):
            base = W1_OFF + 1024 * h + 128 * k
            return wpack[:, base:base + 128]

        def a_pair(layer, h):
            base = A_OFF + 2 * (H * layer + h)
            return wpack[:, base:base + 2]

        # adj^T via DMA XBAR transpose: adjT_all[g][p, 512*j + n] = adj[g][n, 128j+p]
        adjT_all = [pers.tile([128, NCH * N], BF16, tag=f"adjT{g}", name=f"adjT{g}")
                    for g in range(G)]
        xTa = [pers.tile([128, N], BF16, tag=f"xTa{g}", name=f"xTa{g}") for g in range(G)]
        xTb = [pers.tile([3, N], BF16, tag=f"xTb{g}", name=f"xTb{g}") for g in range(G)]
        x1T = [[pers.tile([128, N], BF16, tag=f"x1T{g}_{h}", name=f"x1T{g}_{h}") for h in range(H)]
               for g in range(G)]
        acc = [pers.tile([128, N], F32, tag=f"acc{g}", name=f"acc{g}") for g in range(G)]

        def adjT(g, j):
            return adjT_all[g][:, N * j:N * (j + 1)]

        # ---------- stage 1: per-graph preprocessing ----------
        def instance_norm_to(dst, src, P, wb_col):
            # dst[P,N] = (src - mu) * rstd * w + b, stats over free dim
            s1 = wk.tile([P, 1], F32, tag="in_sum")
            nc.vector.tensor_reduce(s1[:], src[:], AX.X, OP.add)
            sq = wk.tile([P, N], BF16, tag="in_sq", name="sq")
            ssq = wk.tile([P, 1], F32, tag="in_ssq")
            nc.scalar.activation(sq[:], src[:], ACT.Square, accum_out=ssq[:])
            mu = wk.tile([P, 1], F32, tag="in_mu")
            nc.vector.tensor_scalar(mu[:], s1[:], 1.0 / N, None, OP.mult)
            ex2 = wk.tile([P, 1], F32, tag="in_ex2")
            nc.vector.tensor_scalar(ex2[:], ssq[:], 1.0 / N, None, OP.mult)
            musq = wk.tile([P, 1], F32, tag="in_musq")
            nc.vector.tensor_tensor(out=musq[:], in0=mu[:], in1=mu[:], op=OP.mult)
            var = wk.tile([P, 1], F32, tag="in_var")
            nc.vector.tensor_tensor(out=var[:], in0=ex2[:], in1=musq[:], op=OP.subtract)
            vare = wk.tile([P, 1], F32, tag="in_vare")
            nc.vector.tensor_scalar(vare[:], var[:], EPS, None, OP.add)
            # rsqrt via quake seed + 3 Newton iterations, all on DVE
            iv = vare[:].bitcast(mybir.dt.int32)
            sh = wk.tile([P, 1], mybir.dt.int32, tag="in_sh")
            nc.vector.tensor_scalar(sh[:], iv, 1, None, OP.arith_shift_right)
            y = wk.tile([P, 1], F32, tag="in_y")
            nc.vector.tensor_scalar(y[:].bitcast(mybir.dt.int32), sh[:], -1,
                                    0x5f3759df, OP.mult, OP.add)
            rstd = y
            for it in range(3):
                y2 = wk.tile([P, 1], F32, tag="in_y2", name="y2")
                nc.vector.tensor_tensor(out=y2[:], in0=rstd[:], in1=rstd[:], op=OP.mult)
                vy2 = wk.tile([P, 1], F32, tag="in_vy2", name="vy2")
                nc.vector.tensor_tensor(out=vy2[:], in0=vare[:], in1=y2[:], op=OP.mult)
                corr = wk.tile([P, 1], F32, tag="in_corr", name="corr")
                nc.vector.tensor_scalar(corr[:], vy2[:], -0.5, 1.5, OP.mult, OP.add)
                ynew = wk.tile([P, 1], F32, tag="in_ynew", name="ynew")
                nc.vector.tensor_tensor(out=ynew[:], in0=rstd[:], in1=corr[:], op=OP.mult)
                rstd = ynew
            scl = wk.tile([P, 1], F32, tag="in_scl")
            nc.vector.tensor_tensor(out=scl[:], in0=rstd[:], in1=wb_col[:, 0:1], op=OP.mult)
            tb = wk.tile([P, 1], F32, tag="in_tb")
            nc.vector.tensor_tensor(out=tb[:], in0=mu[:], in1=scl[:], op=OP.mult)
            bia = wk.tile([P, 1], F32, tag="in_bia")
            nc.vector.tensor_tensor(out=bia[:], in0=wb_col[:, 1:2], in1=tb[:], op=OP.subtract)
            nc.vector.tensor_scalar(dst, src[:], scl[:], bia[:], OP.mult, OP.add)

        def prep_graph(g):
            # adj^T straight from DRAM via the DMA XBAR
            nc.sync.dma_start_transpose(
                adjT_all[g][:].rearrange("p (j n) -> p j n", j=NCH), d_adj[g])
            # h^T pre-transposed on host -> xTa rows 0:64
            nc.sync.dma_start(xTa[g][0:F0, :], d_hT[g])
            # embedding gather -> transpose -> instance norm -> xTa rows 64:128
            embT = wk.tile([D_EMB, N], BF16, tag="embT", name="embT")
            for i in range(NCH):
                idx = wk.tile([128, 1], I32, tag="idx")
                nc.gpsimd.dma_start(idx[:], d_vert[g, 128 * i:128 * (i + 1), :])
                gat = wk.tile([128, D_EMB], BF16, tag="gat")
                nc.gpsimd.indirect_dma_start(
                    out=gat[:], out_offset=None, in_=d_emb[:],
                    in_offset=bass.IndirectOffsetOnAxis(ap=idx[:, :1], axis=0))
                tp = pst.tile([128, 128], BF16, tag="tpb", name="prep_tp")
                nc.tensor.transpose(tp[0:D_EMB, :], gat[:], ident_b[:])
                anycopy(embT[:, 128 * i:128 * (i + 1)], tp[0:D_EMB, :])
            instance_norm_to(xTa[g][F0:F0 + D_EMB, :], embT, D_EMB, n1wb)
            # user_emb^T pre-transposed on host -> instance norm -> xTb
            ueT = wk.tile([3, N], BF16, tag="ueT", name="ueT")
            nc.sync.dma_start(ueT[:], d_ueT[g])
            instance_norm_to(xTb[g][:], ueT, 3, n2wb)

        # ---------- pipelined GAT layer machinery ----------
        # stage A(i): hp matmuls, tanh, hp^T->hpblk transposes, s/d row matmul,
        #             d-row->column DMA, p_row exp + broadcast
        # stage EB(i): exps of d columns, E' tile construction (DVE/GPSIMD)
        # stage B(i):  softmax denominator matmul, value matmuls, epilogue

        def stageA1(layer, g, h):
            hp_ps = psm.tile([128, N], F32, tag="mm", name="hp_ps")
            if layer == 0:
                MM(hp_ps[:], w0a_h(h), xTa[g][:], start=True, stop=False)
                MM(hp_ps[:], w0b_r[:, 128 * h:128 * (h + 1)], xTb[g][:],
                   start=False, stop=True)
            else:
                for k in range(8):
                    MM(hp_ps[:], w1_hk(h, k), x1T[g][k][:],
                       start=(k == 0), stop=(k == 7))
            t0 = wk.tile([128, N], BF16, tag="tt", name="t0")
            hp_sb = wk.tile([128, N], BF16, tag="hpsb", name="hp_sb", bufs=2)
            if layer == 0:
                nc.scalar.activation(t0[:], hp_ps[:], ACT.Tanh)
                nc.vector.tensor_copy(out=hp_sb[:], in_=hp_ps[:])
            else:
                ncs = negcs1[:, h:h + 1]
                nc.scalar.activation(t0[:], hp_ps[:], ACT.Tanh, bias=ncs)
                nc.vector.tensor_scalar(hp_sb[:], hp_ps[:], ncs, None, OP.add)
            return dict(g=g, h=h, layer=layer, t0=t0, hp_sb=hp_sb)

        def stageA2(st):
            layer, g, h = st["layer"], st["g"], st["h"]
            t0, hp_sb = st["t0"], st["hp_sb"]
            hpblk = wk.tile([128, N], BF16, tag="hpblk", name="hpblk", bufs=3)
            for j in range(NCH):
                tp = pst.tile([128, 128], BF16, tag="tpb")
                nc.tensor.transpose(tp[:], hp_sb[:, 128 * j:128 * (j + 1)], ident_b[:])
                anycopy(hpblk[:, 128 * j:128 * (j + 1)], tp[:])
            apr = a_pair(layer, h)
            # s row: s[n] = a_src^T tanh(hp^T)[., n]
            s_ps = pss.tile([1, N], F32, tag="sd", name="s_ps")
            MM(s_ps[:], apr[:, 0:1], t0[:], start=True, stop=True)
            # d columns, directly transposed: dcol[m, j] = a_dst^T tanh(hp^T)[., 128j+m]
            dcol_ps = pss.tile([128, NCH], F32, tag="dc", name="dcol_ps")
            for j in range(NCH):
                MM(dcol_ps[:, j:j + 1], t0[:, 128 * j:128 * (j + 1)], apr[:, 1:2],
                   start=True, stop=True)
            # softmax-row-invariant factor e^{0.8 s[n]} broadcast over partitions
            p_row = wk.tile([1, N], BF16, tag="prow", bufs=2)
            nc.scalar.activation(p_row[:], s_ps[0:1, :], ACT.Exp, scale=0.8)
            p_b = wk.tile([128, N], BF16, tag="pb", bufs=3)
            nc.gpsimd.partition_broadcast(p_b[:], p_row[:])
            st["dcol_ps"] = dcol_ps
            st["p_b"] = p_b
            st["hpblk"] = hpblk
            return st

        def stageEB(st):
            g = st["g"]
            dcol, p_b = st["dcol_ps"], st["p_b"]
            acol = wk.tile([128, NCH], F32, tag="acol", bufs=2)
            nc.scalar.activation(acol[:], dcol[:], ACT.Exp, scale=0.2)
            qacol = wk.tile([128, NCH], F32, tag="qacol", bufs=2)
            nc.scalar.activation(qacol[:], dcol[:], ACT.Exp, scale=1.0)
            eps_ = []
            for j in range(NCH):
                e = ep_pool.tile([128, N], BF16, tag="ep", name="e")
                nc.vector.tensor_scalar(e[:], p_b[:], qacol[:, j:j + 1],
                                        acol[:, j:j + 1], OP.mult, OP.max)
                if j < 3:
                    nc.vector.tensor_tensor(out=e[:], in0=e[:], in1=adjT(g, j),
                                            op=OP.mult)
                else:
                    nc.gpsimd.tensor_tensor(out=e[:], in0=e[:], in1=adjT(g, j),
                                            op=OP.mult)
                eps_.append(e)
            st["eps"] = eps_
            return st

        def stageB(st):
            g, h, layer = st["g"], st["h"], st["layer"]
            hpblk, eps_ = st["hpblk"], st["eps"]
            rs_ps = pss.tile([1, N], F32, tag="sd", name="rs_ps")
            colw = ones_b if layer == 0 else eights_b
            for j in range(NCH):
                MM(rs_ps[:], colw[:], eps_[j][:],
                   start=(j == 0), stop=(j == NCH - 1))
            rrec = wk.tile([1, N], F32, tag="rrec")
            nc.vector.reciprocal_approx_fast(out=rrec[:], in_=rs_ps[:])
            rb = wk.tile([128, N], F32, tag="rb")
            nc.gpsimd.partition_broadcast(rb[:], rrec[:])
            out_ps = psm.tile([128, N], F32, tag="mm", name="out_ps")
            for j in range(NCH):
                MM(out_ps[:], hpblk[:, 128 * j:128 * (j + 1)], eps_[j][:],
                   start=(j == 0), stop=(j == NCH - 1))
            if layer == 0:
                z = wk.tile([128, N], F32, tag="z")
                nc.vector.tensor_tensor(out=z[:], in0=out_ps[:], in1=rb[:], op=OP.mult)
                # elu(z)+1 = min(exp(z), relu(z)+1); -1 folded into layer-1 bias
                ez = wk.tile([128, N], F32, tag="ez")
                nc.scalar.activation(ez[:], z[:], ACT.Exp)
                r1 = wk.tile([128, N], F32, tag="r1")
                nc.vector.tensor_scalar(r1[:], z[:], 0.0, 1.0, OP.max, OP.add)
                nc.vector.tensor_tensor(out=x1T[g][h][:], in0=ez[:], in1=r1[:],
                                        op=OP.min)
            else:
                if h == 0:
                    nc.vector.tensor_tensor(out=acc[g][:], in0=out_ps[:], in1=rb[:],
                                            op=OP.mult)
                else:
                    contrib = wk.tile([128, N], F32, tag="contrib")
                    nc.vector.tensor_tensor(out=contrib[:], in0=out_ps[:], in1=rb[:],
                                            op=OP.mult)
                    nc.vector.tensor_tensor(out=acc[g][:], in0=acc[g][:],
                                            in1=contrib[:], op=OP.add)

        # ---------- run the pipeline across both layers ----------
        prep_graph(0)
        steps = [(l, g, h) for l in range(2) for g in range(G) for h in range(H)]
        window = []  # sts with A done, EB pending/done
        first = True
        for (l, g, h) in steps:
            st = stageA1(l, g, h)
            if first:
                prep_graph(1)  # overlaps with layer-0 compute on graph 0
                first = False
            if len(window) >= 2:
                stageB(window[0])
                window.pop(0)
            stageA2(st)
            if len(window) >= 1:
                stageEB(window[-1])
            window.append(st)
        stageEB(window[-1])
        for w in window:
            stageB(w)

        # ---------- log_softmax + output ----------
        ztps, nmaxs, sexps = [], [], []
        for g in range(G):
            for j in range(NCH):
                ztp = psm.tile([128, 128], F32, tag="mm", name="ztp")
                nc.tensor.transpose(ztp[:], acc[g][:, 128 * j:128 * (j + 1)], ident[:])
                zsb = wk.tile([128, 128], F32, tag="zsb", bufs=8, name="zsb")
                nc.vector.tensor_copy(out=zsb[:], in_=ztp[:])
                nmax = wk.tile([128, 1], F32, tag="nmax", bufs=8, name="nmax")
                nc.vector.tensor_reduce(nmax[:], zsb[:], AX.X, OP.max, negate=True)
                esc = wk.tile([128, 128], F32, tag="esc", name="esc")
                sexp = wk.tile([128, 1], F32, tag="sexp", bufs=8, name="sexp")
                nc.scalar.activation(esc[:], zsb[:], ACT.Exp, bias=nmax[:],
                                     accum_out=sexp[:])
                ztps.append(zsb); nmaxs.append(nmax); sexps.append(sexp)
        fins = []
        for i in range(G * NCH):
            lns = wk.tile([128, 1], F32, tag="lns", bufs=8, name="lns")
            nc.scalar.activation(lns[:], sexps[i][:], ACT.Ln)
            cc = wk.tile([128, 1], F32, tag="cc", bufs=8, name="cc")
            nc.vector.tensor_tensor(out=cc[:], in0=nmaxs[i][:], in1=lns[:],
                                    op=OP.subtract)
            fin = wk.tile([128, 128], F32, tag="fin", bufs=4, name="fin")
            nc.vector.tensor_scalar(fin[:], ztps[i][:], cc[:], None, OP.add)
            fins.append(fin)
        i = 0
        for g in range(G):
            for j in range(NCH):
                nc.sync.dma_start(d_out[g, 128 * j:128 * (j + 1), :], fins[i][:])
                i += 1

    nc.finalize()
    return nc


def _get_nc():
    if "nc" not in _cache:
        _cache["nc"] = _build_nc()
    return _cache["nc"]


def shard_inputs(inputs):
    """Full inputs -> list of 8 per-core input maps (host-side layout+cast)."""
    import ml_dtypes
    bf16 = ml_dtypes.bfloat16

    vertices = np.asarray(inputs["vertices"]).astype(np.int32).reshape(B, N, 1)
    adj = np.asarray(inputs["adj"], dtype=np.float32).astype(bf16)
    hT = np.ascontiguousarray(
        np.asarray(inputs["h"], dtype=np.float32).transpose(0, 2, 1)).astype(bf16)
    ueT = np.ascontiguousarray(
        np.asarray(inputs["user_emb"], dtype=np.float32).transpose(0, 2, 1)).astype(bf16)
    emb = np.asarray(inputs["emb_table"], dtype=np.float32).astype(bf16)

    w0 = np.asarray(inputs["w0"], dtype=np.float32)       # [H, 131, 128]
    w1 = np.asarray(inputs["w1"], dtype=np.float32)       # [H, 1024, 128]
    a0s = np.asarray(inputs["a_src0"], dtype=np.float32)[..., 0]  # [H, 128]
    a0d = np.asarray(inputs["a_dst0"], dtype=np.float32)[..., 0]
    a1s = np.asarray(inputs["a_src1"], dtype=np.float32)[..., 0]
    a1d = np.asarray(inputs["a_dst1"], dtype=np.float32)[..., 0]

    wpack = np.zeros((128, WPACK_COLS), dtype=np.float32)
    for h in range(H):
        wpack[:, W0A_OFF + 128 * h:W0A_OFF + 128 * (h + 1)] = w0[h, :128, :]
        for k in range(8):
            base = W1_OFF + 1024 * h + 128 * k
            wpack[:, base:base + 128] = w1[h, 128 * k:128 * (k + 1), :]
        wpack[:, A_OFF + 2 * h] = a0s[h]
        wpack[:, A_OFF + 2 * h + 1] = a0d[h]
        wpack[:, A_OFF + 2 * H + 2 * h] = a1s[h]
        wpack[:, A_OFF + 2 * H + 2 * h + 1] = a1d[h]
    wpack = wpack.astype(bf16)
    w0b = np.zeros((3, H * 128), dtype=np.float32)
    for h in range(H):
        w0b[:, 128 * h:128 * (h + 1)] = w0[h, 128:, :]
    w0b = w0b.astype(bf16)

    # elu fold bias: x1 is stored as elu(.)+1, so layer-1 needs -colsum(w1) per head.
    # matmul runs in bf16, so compute the colsum from the bf16-rounded weights.
    ncs = -w1.astype(bf16).astype(np.float32).sum(axis=1).transpose(1, 0)  # [128, H]
    ncs = np.ascontiguousarray(ncs, dtype=np.float32)

    n1wb = np.stack([np.asarray(inputs["norm1_w"], dtype=np.float32),
                     np.asarray(inputs["norm1_b"], dtype=np.float32)], axis=1)
    n2wb = np.stack([np.asarray(inputs["norm2_w"], dtype=np.float32),
                     np.asarray(inputs["norm2_b"], dtype=np.float32)], axis=1)

    maps = []
    for c in range(NCORES):
        sl = slice(G * c, G * (c + 1))
        maps.append({
            "vertices": np.ascontiguousarray(vertices[sl]),
            "adj": np.ascontiguousarray(adj[sl]),
            "hT": np.ascontiguousarray(hT[sl]),
            "ueT": np.ascontiguousarray(ueT[sl]),
            "emb": emb,
            "wpack": wpack, "w0b": w0b, "ncs": ncs,
            "n1wb": n1wb, "n2wb": n2wb,
        })
    return maps


def kernel(**inputs):
    _ensure_paths()
    from concourse import bass_utils
    nc = _get_nc()
    maps = shard_inputs(inputs)
    res = bass_utils.run_bass_kernel_spmd(nc, maps, core_ids=list(range(NCORES)))
    out = np.concatenate([res.results[c]["out"] for c in range(NCORES)], axis=0)
    return out


# revision 10
# speedup vs baseline: 2.2416x; 2.2067x over previous
"""Trainium2 Bass kernel for nn_BatchdenseGAT: 2-layer dense GAT, batch 16x512 nodes.

Strategy: pure data-parallel over the batch dim -- 2 graphs per NeuronCore, 8 cores.
Each core runs embedding gather + instance norms + 2 GAT layers + log_softmax for its
2 graphs. No collectives.

v2 notes (vs baseline):
  - All activations/weights pre-cast to bf16 on host; h and user_emb pre-transposed
    on host; weights packed into one [128, C] DRAM tensor; -colsum(w1) (the elu+1
    bias fold) precomputed on host. No device-side casts / weight staging.
  - adj transposed by the DMA XBAR engine (dma_start_transpose) straight from DRAM,
    eliminating 32 fp32 PE transposes + psum->sbuf copies per core.
  - attention s/d rows: d-row is redistributed into per-chunk columns with one small
    SBUF->SBUF DMA instead of 4 PE transposes per (g,h,layer).
  - software pipeline depth 2 (A(i) ... EB(i-1) ... B(i-2)) to hide the dcol DMA
    latency and keep the tensor engine dense (p-state ramp).
  - DMA triggers ride the sync queue; elementwise work balanced across DVE/ACT/GPSIMD.

Layout (per graph, per head):
  - x^T [131, 512] feature-major feeds matmuls as the moving operand.
  - hp^T [128, 512] from weights-stationary matmuls.
  - attention built in TRANSPOSED form E'[m, n] (m=src, n=dst); E' blocks are the
    moving operand of attn@hp with contraction over m on partitions.
  - exp(leaky_relu(s[n]+d[m], .2)) = e^{.2s[n]} * max(e^{.8s[n]}e^{d[m]}, e^{.2d[m]});
    the e^{.2s[n]} column factor is softmax-invariant and dropped.
  - softmax denominators via ones-column matmuls; normalization in epilogues.
  - elu(z)+1 = min(exp(z), relu(z)+1); the -1 is folded into layer 1 via the
    host-computed -colsum(w1) bias.
"""

import os
import sys
import numpy as np

B, N, V, D_EMB, F0, H = 16, 512, 100000, 64, 64, 8
O1 = O2 = 128
F_IN0 = F0 + D_EMB + 3  # 131
F_IN1 = H * O1          # 1024
EPS = 1e-5
NCORES = 8
G = B // NCORES         # graphs per core = 2
NCH = N // 128          # 4 node chunks

# packed weight tensor column offsets (bf16, 128 partitions)
W0A_OFF = 0                      # 8 heads x 128 cols
W1_OFF = W0A_OFF + H * 128       # 8 heads x 8 ktiles x 128 cols
A_OFF = W1_OFF + H * 1024        # 2 layers x 8 heads x 2 cols (src,dst)
WPACK_COLS = A_OFF + 4 * H

_cache = {}


def _ensure_paths():
    p = "/opt/trn_rl_repo/concourse"
    if os.path.isdir(p) and p not in sys.path:
        sys.path.append(p)


def _build_nc():
    _ensure_paths()
    import concourse.bass as bass
    import concourse.tile as tile
    import concourse.mybir as mybir
    from concourse import bacc
    from concourse.masks import make_identity
    from contextlib import ExitStack

    F32 = mybir.dt.float32
    BF16 = mybir.dt.bfloat16
    I32 = mybir.dt.int32
    AX = mybir.AxisListType
    OP = mybir.AluOpType
    ACT = mybir.ActivationFunctionType

    nc = bacc.Bacc("TRN2", debug=False, enable_asserts=False)

    d_vert = nc.dram_tensor("vertices", [G, N, 1], I32, kind="ExternalInput").ap()
    d_adj = nc.dram_tensor("adj", [G, N, N], BF16, kind="ExternalInput").ap()
    d_hT = nc.dram_tensor("hT", [G, F0, N], BF16, kind="ExternalInput").ap()
    d_ueT = nc.dram_tensor("ueT", [G, 3, N], BF16, kind="ExternalInput").ap()
    d_emb = nc.dram_tensor("emb", [V, D_EMB], BF16, kind="ExternalInput").ap()
    d_wpack = nc.dram_tensor("wpack", [128, WPACK_COLS], BF16, kind="ExternalInput").ap()
    d_w0b = nc.dram_tensor("w0b", [3, H * 128], BF16, kind="ExternalInput").ap()
    d_ncs = nc.dram_tensor("ncs", [128, H], F32, kind="ExternalInput").ap()
    d_n1wb = nc.dram_tensor("n1wb", [D_EMB, 2], F32, kind="ExternalInput").ap()
    d_n2wb = nc.dram_tensor("n2wb", [3, 2], F32, kind="ExternalInput").ap()
    d_out = nc.dram_tensor("out", [G, N, O2], F32, kind="ExternalOutput").ap()

    with tile.TileContext(nc) as tc, ExitStack() as ctx:
        pers = ctx.enter_context(tc.tile_pool(name="pers", bufs=1))
        wk = ctx.enter_context(tc.tile_pool(name="wk", bufs=2))
        ep_pool = ctx.enter_context(tc.tile_pool(name="ep", bufs=12))
        psm = ctx.enter_context(tc.tile_pool(name="psm", bufs=2, space="PSUM"))
        pst = ctx.enter_context(tc.tile_pool(name="pst", bufs=2, space="PSUM"))
        pss = ctx.enter_context(tc.tile_pool(name="pss", bufs=2, space="PSUM"))

        _cp = [0]

        def anycopy(out, in_):
            # alternate ACT/DVE for psum->sbuf copies (GPSIMD cannot read PSUM)
            _cp[0] ^= 1
            if _cp[0]:
                nc.scalar.copy(out, in_)
            else:
                nc.vector.tensor_copy(out=out, in_=in_)

        MM = nc.tensor.matmul

        # ---------- persistents ----------
        ident_b = pers.tile([128, 128], BF16, tag="ident_b")
        make_identity(nc, ident_b[:])
        ident = pers.tile([128, 128], F32, tag="ident")
        make_identity(nc, ident[:])

        ones_b = pers.tile([128, 1], BF16, tag="ones_b")
        nc.vector.memset(ones_b[:], 1.0)
        eights_b = pers.tile([128, 1], BF16, tag="eights_b")
        nc.vector.memset(eights_b[:], float(H))

        # packed weights -> SBUF directly (bf16, no staging/cast)
        wpack = pers.tile([128, WPACK_COLS], BF16, tag="wpack")
        NSPLIT = 4
        cw = WPACK_COLS // NSPLIT
        for s in range(NSPLIT):
            nc.sync.dma_start(wpack[:, cw * s:cw * (s + 1)],
                              d_wpack[:, cw * s:cw * (s + 1)])
        w0b_r = pers.tile([3, H * 128], BF16, tag="w0b_r")
        nc.sync.dma_start(w0b_r[:], d_w0b[:])
        negcs1 = pers.tile([128, H], F32, tag="negcs1")
        nc.sync.dma_start(negcs1[:], d_ncs[:])
        n1wb = pers.tile([D_EMB, 2], F32, tag="n1wb")
        nc.sync.dma_start(n1wb[:], d_n1wb[:])
        n2wb = pers.tile([3, 2], F32, tag="n2wb")
        nc.sync.dma_start(n2wb[:], d_n2wb[:])

        def w0a_h(h):
            return wpack[:, W0A_OFF + 128 * h:W0A_OFF + 128 * (h + 1)]

        def w1_hk(h, k):
            base = W1_OFF + 1024 * h + 128 * k
            return wpack[:, base:base + 128]

        def a_pair(layer, h):
            base = A_OFF + 2 * (H * layer + h)
            return wpack[:, base:base + 2]

        # adj^T via DMA XBAR transpose: adjT_all[g][p, 512*j + n] = adj[g][n, 128j+p]
        adjT_all = [pers.tile([128, NCH * N], BF16, tag=f"adjT{g}", name=f"adjT{g}")
                    for g in range(G)]
        xTa = [pers.tile([128, N], BF16, tag=f"xTa{g}", name=f"xTa{g}") for g in range(G)]
        xTb = [pers.tile([3, N], BF16, tag=f"xTb{g}", name=f"xTb{g}") for g in range(G)]
        x1T = [[pers.tile([128, N], BF16, tag=f"x1T{g}_{h}", name=f"x1T{g}_{h}") for h in range(H)]
               for g in range(G)]
        acc = [pers.tile([128, N], F32, tag=f"acc{g}", name=f"acc{g}") for g in range(G)]

        def adjT(g, j):
            return adjT_all[g][:, N * j:N * (j + 1)]

        # ---------- stage 1: per-graph preprocessing ----------
        def instance_norm_to(dst, src, P, wb_col):
            # dst[P,N] = (src - mu) * rstd * w + b, stats over free dim
            s1 = wk.tile([P, 1], F32, tag="in_sum")
            nc.vector.tensor_reduce(s1[:], src[:], AX.X, OP.add)
            sq = wk.tile([P, N], BF16, tag="in_sq", name="sq")
            ssq = wk.tile([P, 1], F32, tag="in_ssq")
            nc.scalar.activation(sq[:], src[:], ACT.Square, accum_out=ssq[:])
            mu = wk.tile([P, 1], F32, tag="in_mu")
            nc.vector.tensor_scalar(mu[:], s1[:], 1.0 / N, None, OP.mult)
            ex2 = wk.tile([P, 1], F32, tag="in_ex2")
            nc.vector.tensor_scalar(ex2[:], ssq[:], 1.0 / N, None, OP.mult)
            musq = wk.tile([P, 1], F32, tag="in_musq")
            nc.vector.tensor_tensor(out=musq[:], in0=mu[:], in1=mu[:], op=OP.mult)
            var = wk.tile([P, 1], F32, tag="in_var")
            nc.vector.tensor_tensor(out=var[:], in0=ex2[:], in1=musq[:], op=OP.subtract)
            vare = wk.tile([P, 1], F32, tag="in_vare")
            nc.vector.tensor_scalar(vare[:], var[:], EPS, None, OP.add)
            # rsqrt via quake seed + 3 Newton iterations, all on DVE
            iv = vare[:].bitcast(mybir.dt.int32)
            sh = wk.tile([P, 1], mybir.dt.int32, tag="in_sh")
            nc.vector.tensor_scalar(sh[:], iv, 1, None, OP.arith_shift_right)
            y = wk.tile([P, 1], F32, tag="in_y")
            nc.vector.tensor_scalar(y[:].bitcast(mybir.dt.int32), sh[:], -1,
                                    0x5f3759df, OP.mult, OP.add)
            rstd = y
            for it in range(3):
                y2 = wk.tile([P, 1], F32, tag="in_y2", name="y2")
                nc.vector.tensor_tensor(out=y2[:], in0=rstd[:], in1=rstd[:], op=OP.mult)
                vy2 = wk.tile([P, 1], F32, tag="in_vy2", name="vy2")
                nc.vector.tensor_tensor(out=vy2[:], in0=vare[:], in1=y2[:], op=OP.mult)
                corr = wk.tile([P, 1], F32, tag="in_corr", name="corr")
                nc.vector.tensor_scalar(corr[:], vy2[:], -0.5, 1.5, OP.mult, OP.add)
                ynew = wk.tile([P, 1], F32, tag="in_ynew", name="ynew")
                nc.vector.tensor_tensor(out=ynew[:], in0=rstd[:], in1=corr[:], op=OP.mult)
                rstd = ynew
            scl = wk.tile([P, 1], F32, tag="in_scl")
            nc.vector.tensor_tensor(out=scl[:], in0=rstd[:], in1=wb_col[:, 0:1], op=OP.mult)
            tb = wk.tile([P, 1], F32, tag="in_tb")
            nc.vector.tensor_tensor(out=tb[:], in0=mu[:], in1=scl[:], op=OP.mult)
            bia = wk.tile([P, 1], F32, tag="in_bia")
            nc.vector.tensor_tensor(out=bia[:], in0=wb_col[:, 1:2], in1=tb[:], op=OP.subtract)
            nc.vector.tensor_scalar(dst, src[:], scl[:], bia[:], OP.mult, OP.add)

        def prep_graph(g):
            # adj^T straight from DRAM via the DMA XBAR
            nc.sync.dma_start_transpose(
                adjT_all[g][:].rearrange("p (j n) -> p j n", j=NCH), d_adj[g])
            # h^T pre-transposed on host -> xTa rows 0:64
            nc.sync.dma_start(xTa[g][0:F0, :], d_hT[g])
            # embedding gather -> transpose -> instance norm -> xTa rows 64:128
            embT = wk.tile([D_EMB, N], BF16, tag="embT", name="embT")
            for i in range(NCH):
                idx = wk.tile([128, 1], I32, tag="idx")
                nc.gpsimd.dma_start(idx[:], d_vert[g, 128 * i:128 * (i + 1), :])
                gat = wk.tile([128, D_EMB], BF16, tag="gat")
                nc.gpsimd.indirect_dma_start(
                    out=gat[:], out_offset=None, in_=d_emb[:],
                    in_offset=bass.IndirectOffsetOnAxis(ap=idx[:, :1], axis=0))
                tp = pst.tile([128, 128], BF16, tag="tpb", name="prep_tp")
                nc.tensor.transpose(tp[0:D_EMB, :], gat[:], ident_b[:])
                anycopy(embT[:, 128 * i:128 * (i + 1)], tp[0:D_EMB, :])
            instance_norm_to(xTa[g][F0:F0 + D_EMB, :], embT, D_EMB, n1wb)
            # user_emb^T pre-transposed on host -> instance norm -> xTb
            ueT = wk.tile([3, N], BF16, tag="ueT", name="ueT")
            nc.sync.dma_start(ueT[:], d_ueT[g])
            instance_norm_to(xTb[g][:], ueT, 3, n2wb)

        # ---------- pipelined GAT layer machinery ----------
        # stage A(i): hp matmuls, tanh, hp^T->hpblk transposes, s/d row matmul,
        #             d-row->column DMA, p_row exp + broadcast
        # stage EB(i): exps of d columns, E' tile construction (DVE/GPSIMD)
        # stage B(i):  softmax denominator matmul, value matmuls, epilogue

        def stageA1(layer, g, h):
            hp_ps = psm.tile([128, N], F32, tag="mm", name="hp_ps")
            if layer == 0:
                MM(hp_ps[:], w0a_h(h), xTa[g][:], start=True, stop=False)
                MM(hp_ps[:], w0b_r[:, 128 * h:128 * (h + 1)], xTb[g][:],
                   start=False, stop=True)
            else:
                for k in range(8):
                    MM(hp_ps[:], w1_hk(h, k), x1T[g][k][:],
                       start=(k == 0), stop=(k == 7))
            t0 = wk.tile([128, N], BF16, tag="tt", name="t0")
            hp_sb = wk.tile([128, N], BF16, tag="hpsb", name="hp_sb", bufs=2)
            if layer == 0:
                nc.scalar.activation(t0[:], hp_ps[:], ACT.Tanh)
                nc.vector.tensor_copy(out=hp_sb[:], in_=hp_ps[:])
            else:
                ncs = negcs1[:, h:h + 1]
                nc.scalar.activation(t0[:], hp_ps[:], ACT.Tanh, bias=ncs)
                nc.vector.tensor_scalar(hp_sb[:], hp_ps[:], ncs, None, OP.add)
            return dict(g=g, h=h, layer=layer, t0=t0, hp_sb=hp_sb)

        def stageA2(st):
            layer, g, h = st["layer"], st["g"], st["h"]
            t0, hp_sb = st["t0"], st["hp_sb"]
            hpblk = wk.tile([128, N], BF16, tag="hpblk", name="hpblk", bufs=3)
            for j in range(NCH):
                tp = pst.tile([128, 128], BF16, tag="tpb")
                nc.tensor.transpose(tp[:], hp_sb[:, 128 * j:128 * (j + 1)], ident_b[:])
                anycopy(hpblk[:, 128 * j:128 * (j + 1)], tp[:])
            apr = a_pair(layer, h)
            # s row: s[n] = a_src^T tanh(hp^T)[., n]
            s_ps = pss.tile([1, N], F32, tag="sd", name="s_ps")
            MM(s_ps[:], apr[:, 0:1], t0[:], start=True, stop=True)
            # d columns, directly transposed: dcol[m, j] = a_dst^T tanh(hp^T)[., 128j+m]
            dcol_ps = pss.tile([128, NCH], F32, tag="dc", name="dcol_ps")
            for j in range(NCH):
                MM(dcol_ps[:, j:j + 1], t0[:, 128 * j:128 * (j + 1)], apr[:, 1:2],
                   start=True, stop=True)
            # softmax-row-invariant factor e^{0.8 s[n]} broadcast over partitions
            p_row = wk.tile([1, N], BF16, tag="prow", bufs=2)
            nc.scalar.activation(p_row[:], s_ps[0:1, :], ACT.Exp, scale=0.8)
            p_b = wk.tile([128, N], BF16, tag="pb", bufs=3)
            nc.gpsimd.partition_broadcast(p_b[:], p_row[:])
            st["dcol_ps"] = dcol_ps
            st["p_b"] = p_b
            st["hpblk"] = hpblk
            return st

        def stageEB(st):
            g = st["g"]
            dcol, p_b = st["dcol_ps"], st["p_b"]
            acol = wk.tile([128, NCH], F32, tag="acol", bufs=2)
            nc.scalar.activation(acol[:], dcol[:], ACT.Exp, scale=0.2)
            qacol = wk.tile([128, NCH], F32, tag="qacol", bufs=2)
            nc.scalar.activation(qacol[:], dcol[:], ACT.Exp, scale=1.0)
            eps_ = []
            for j in range(NCH):
                # U = p_b * e^{d[m]} on ACT; then (U max acol) * adjT fused on DVE
                u = wk.tile([128, N], BF16, tag="ub", name="u", bufs=2)
                nc.scalar.activation(u[:], p_b[:], ACT.Copy, scale=qacol[:, j:j + 1])
                e = ep_pool.tile([128, N], BF16, tag="ep", name="e")
                nc.vector.scalar_tensor_tensor(
                    out=e[:], in0=u[:], scalar=acol[:, j:j + 1], in1=adjT(g, j),
                    op0=OP.max, op1=OP.mult)
                eps_.append(e)
            st["eps"] = eps_
            return st

        def stageB(st):
            g, h, layer = st["g"], st["h"], st["layer"]
            hpblk, eps_ = st["hpblk"], st["eps"]
            rs_ps = pss.tile([1, N], F32, tag="sd", name="rs_ps")
            colw = ones_b if layer == 0 else eights_b
            for j in range(NCH):
                MM(rs_ps[:], colw[:], eps_[j][:],
                   start=(j == 0), stop=(j == NCH - 1))
            rrec = wk.tile([1, N], F32, tag="rrec")
            nc.vector.reciprocal_approx_fast(out=rrec[:], in_=rs_ps[:])
            rb = wk.tile([128, N], F32, tag="rb")
            nc.gpsimd.partition_broadcast(rb[:], rrec[:])
            out_ps = psm.tile([128, N], F32, tag="mm", name="out_ps")
            for j in range(NCH):
                MM(out_ps[:], hpblk[:, 128 * j:128 * (j + 1)], eps_[j][:],
                   start=(j == 0), stop=(j == NCH - 1))
            if layer == 0:
                z = wk.tile([128, N], F32, tag="z")
                nc.vector.tensor_tensor(out=z[:], in0=out_ps[:], in1=rb[:], op=OP.mult)
                # elu(z)+1 = min(exp(z), relu(z)+1); -1 folded into layer-1 bias
                ez = wk.tile([128, N], F32, tag="ez")
                nc.scalar.activation(ez[:], z[:], ACT.Exp)
                r1 = wk.tile([128, N], F32, tag="r1")
                nc.vector.tensor_scalar(r1[:], z[:], 0.0, 1.0, OP.max, OP.add)
                nc.vector.tensor_tensor(out=x1T[g][h][:], in0=ez[:], in1=r1[:],
                                        op=OP.min)
            else:
                if h == 0:
                    nc.vector.tensor_tensor(out=acc[g][:], in0=out_ps[:], in1=rb[:],
                                            op=OP.mult)
                else:
                    contrib = wk.tile([128, N], F32, tag="contrib")
                    nc.vector.tensor_tensor(out=contrib[:], in0=out_ps[:], in1=rb[:],
                                            op=OP.mult)
                    nc.vector.tensor_tensor(out=acc[g][:], in0=acc[g][:],
                                            in1=contrib[:], op=OP.add)

        # ---------- run the pipeline across both layers ----------
        prep_graph(0)
        steps = [(l, g, h) for l in range(2) for g in range(G) for h in range(H)]
        window = []  # sts with A done, EB pending/done
        first = True
        for (l, g, h) in steps:
            st = stageA1(l, g, h)
            if first:
                prep_graph(1)  # overlaps with layer-0 compute on graph 0
                first = False
            if len(window) >= 2:
                stageB(window[0])
                window.pop(0)
            stageA2(st)
            if len(window) >= 1:
                stageEB(window[-1])
            window.append(st)
        stageEB(window[-1])
        for w in window:
            stageB(w)

        # ---------- log_softmax + output ----------
        ztps, nmaxs, sexps = [], [], []
        for g in range(G):
            for j in range(NCH):
                ztp = psm.tile([128, 128], F32, tag="mm", name="ztp")
                nc.tensor.transpose(ztp[:], acc[g][:, 128 * j:128 * (j + 1)], ident[:])
                zsb = wk.tile([128, 128], F32, tag="zsb", bufs=8, name="zsb")
                nc.vector.tensor_copy(out=zsb[:], in_=ztp[:])
                nmax = wk.tile([128, 1], F32, tag="nmax", bufs=8, name="nmax")
                nc.vector.tensor_reduce(nmax[:], zsb[:], AX.X, OP.max, negate=True)
                esc = wk.tile([128, 128], F32, tag="esc", name="esc")
                sexp = wk.tile([128, 1], F32, tag="sexp", bufs=8, name="sexp")
                nc.scalar.activation(esc[:], zsb[:], ACT.Exp, bias=nmax[:],
                                     accum_out=sexp[:])
                ztps.append(zsb); nmaxs.append(nmax); sexps.append(sexp)
        fins = []
        for i in range(G * NCH):
            lns = wk.tile([128, 1], F32, tag="lns", bufs=8, name="lns")
            nc.scalar.activation(lns[:], sexps[i][:], ACT.Ln)
            cc = wk.tile([128, 1], F32, tag="cc", bufs=8, name="cc")
            nc.vector.tensor_tensor(out=cc[:], in0=nmaxs[i][:], in1=lns[:],
                                    op=OP.subtract)
            fin = wk.tile([128, 128], F32, tag="fin", bufs=4, name="fin")
            nc.vector.tensor_scalar(fin[:], ztps[i][:], cc[:], None, OP.add)
            fins.append(fin)
        i = 0
        for g in range(G):
            for j in range(NCH):
                nc.sync.dma_start(d_out[g, 128 * j:128 * (j + 1), :], fins[i][:])
                i += 1

    nc.finalize()
    return nc


def _get_nc():
    if "nc" not in _cache:
        _cache["nc"] = _build_nc()
    return _cache["nc"]


def shard_inputs(inputs):
    """Full inputs -> list of 8 per-core input maps (host-side layout+cast)."""
    import ml_dtypes
    bf16 = ml_dtypes.bfloat16

    vertices = np.asarray(inputs["vertices"]).astype(np.int32).reshape(B, N, 1)
    adj = np.asarray(inputs["adj"], dtype=np.float32).astype(bf16)
    hT = np.ascontiguousarray(
        np.asarray(inputs["h"], dtype=np.float32).transpose(0, 2, 1)).astype(bf16)
    ueT = np.ascontiguousarray(
        np.asarray(inputs["user_emb"], dtype=np.float32).transpose(0, 2, 1)).astype(bf16)
    emb = np.asarray(inputs["emb_table"], dtype=np.float32).astype(bf16)

    w0 = np.asarray(inputs["w0"], dtype=np.float32)       # [H, 131, 128]
    w1 = np.asarray(inputs["w1"], dtype=np.float32)       # [H, 1024, 128]
    a0s = np.asarray(inputs["a_src0"], dtype=np.float32)[..., 0]  # [H, 128]
    a0d = np.asarray(inputs["a_dst0"], dtype=np.float32)[..., 0]
    a1s = np.asarray(inputs["a_src1"], dtype=np.float32)[..., 0]
    a1d = np.asarray(inputs["a_dst1"], dtype=np.float32)[..., 0]

    wpack = np.zeros((128, WPACK_COLS), dtype=np.float32)
    for h in range(H):
        wpack[:, W0A_OFF + 128 * h:W0A_OFF + 128 * (h + 1)] = w0[h, :128, :]
        for k in range(8):
            base = W1_OFF + 1024 * h + 128 * k
            wpack[:, base:base + 128] = w1[h, 128 * k:128 * (k + 1), :]
        wpack[:, A_OFF + 2 * h] = a0s[h]
        wpack[:, A_OFF + 2 * h + 1] = a0d[h]
        wpack[:, A_OFF + 2 * H + 2 * h] = a1s[h]
        wpack[:, A_OFF + 2 * H + 2 * h + 1] = a1d[h]
    wpack = wpack.astype(bf16)
    w0b = np.zeros((3, H * 128), dtype=np.float32)
    for h in range(H):
        w0b[:, 128 * h:128 * (h + 1)] = w0[h, 128:, :]
    w0b = w0b.astype(bf16)

    # elu fold bias: x1 is stored as elu(.)+1, so layer-1 needs -colsum(w1) per head.
    # matmul runs in bf16, so compute the colsum from the bf16-rounded weights.
    ncs = -w1.astype(bf16).astype(np.float32).sum(axis=1).transpose(1, 0)  # [128, H]
    ncs = np.ascontiguousarray(ncs, dtype=np.float32)

    n1wb = np.stack([np.asarray(inputs["norm1_w"], dtype=np.float32),
                     np.asarray(inputs["norm1_b"], dtype=np.float32)], axis=1)
    n2wb = np.stack([np.asarray(inputs["norm2_w"], dtype=np.float32),
                     np.asarray(inputs["norm2_b"], dtype=np.float32)], axis=1)

    maps = []
    for c in range(NCORES):
        sl = slice(G * c, G * (c + 1))
        maps.append({
            "vertices": np.ascontiguousarray(vertices[sl]),
            "adj": np.ascontiguousarray(adj[sl]),
            "hT": np.ascontiguousarray(hT[sl]),
            "ueT": np.ascontiguousarray(ueT[sl]),
            "emb": emb,
            "wpack": wpack, "w0b": w0b, "ncs": ncs,
            "n1wb": n1wb, "n2wb": n2wb,
        })
    return maps


def kernel(**inputs):
    _ensure_paths()
    from concourse import bass_utils
    nc = _get_nc()
    maps = shard_inputs(inputs)
    res = bass_utils.run_bass_kernel_spmd(nc, maps, core_ids=list(range(NCORES)))
    out = np.concatenate([res.results[c]["out"] for c in range(NCORES)], axis=0)
    return out
